# revision 4
# baseline (speedup 1.0000x reference)
"""CombinedAttentionProcessor kernel for 8 Trainium2 NeuronCores.

Problem: B=2, S=4096, C=640, H=8 heads, D=80 head_dim.
    q/k/v = hs @ W{q,k,v}.T ; per-(b,h): softmax(q k^T / sqrt(D)) v ;
    out = attn @ Wo.T + bo + residual.

Sharding: 16 (batch, head) groups -> 2 per core (batch-parallel over B,
head-parallel over H). Each core computes its 2 heads' full attention and a
partial output projection [S, C]; the host sums the 4 partials per batch and
adds bias + residual.

Per-core dataflow (matmuls in fp32r = full PE rate at moving dim >= 256;
probabilities and V in bf16):
  Phase A: load hsT [C, S]; project qT/kT [D, S] (d-major) and v [S, D]
           (natural, with a ones column at position 96 for the softmax
           row-sum; the V moving operand is host-padded to 256 so the fp32r
           matmul stays in its 1-cycle/row regime).
  Phase B: per head, per 512-query chunk: scoresT tiles [128 keys, 512 q]
           on PE; exp(scale*x) on ScalarE (PSUM->SBUF, bf16); AV matmul
           accumulates out_avT [97, 512] over the 32 key tiles -- row 96 is
           the softmax denominator. Normalize with reciprocal + a broadcast
           matmul (ones[128,80] row-0 one-hot) + DVE multiply.
  Phase C: output projection per 128-query tile, accumulated in PSUM over
           both heads, staged into SBUF quarter-buffers and written with 4
           large DMAs.

All DRAM I/O uses host-prepared partition-major layouts so each DMA is 128
contiguous per-partition descriptors (the DMA sequencer's per-descriptor
issue cost would otherwise dominate). fp32r matmuls admit only ONE sync
wait; dummy matmuls right after the input DMAs make PE observe every
DMA-queue semaphore once, and engine assignment keeps every real matmul's
unobserved waits on a single semaphore.
"""
import sys

if "/opt/trn_rl_repo" not in sys.path:
    sys.path.insert(0, "/opt/trn_rl_repo")

import numpy as np

B, S, C = 2, 4096, 640
H, D = 8, 80
HPC = 2          # heads per core
NCORES = 8
KC = C // 128    # 5 contraction tiles over C
WVN = 256        # v-projection moving width (160 data + zero pad)
WON = 768        # wo moving width (640 data + zero pad; keeps fp32r at 1 cyc/row)
SCALE = 1.0 / float(np.sqrt(D))

_NC_CACHE = {}


def build_nc(s=S):
    import concourse.bacc as bacc
    import concourse.mybir as mybir
    import concourse.tile as tile
    from concourse.tile import add_dep_helper

    f32 = mybir.dt.float32
    f32r = mybir.dt.float32r
    bf16 = mybir.dt.bfloat16

    njt = s // 128   # key tiles
    nit = s // 128   # output i-tiles
    nch = s // 512   # query chunks
    qsize = max(1, nit // 16)  # i-tiles per output staging buffer
    assert s % 512 == 0 and (s // 128) % 4 == 0

    nc = bacc.Bacc("TRN2", target_bir_lowering=False, debug=False,
                   num_devices=NCORES)

    # all inputs partition-major, host-prepared (incl. padding)
    hsT = nc.dram_tensor("hsT", [128, KC * s], f32r, kind="ExternalInput")
    wq = nc.dram_tensor("wq", [128, KC * HPC * D], f32r,
                        kind="ExternalInput")
    wk = nc.dram_tensor("wk", [128, KC * HPC * D], f32r,
                        kind="ExternalInput")
    wv = nc.dram_tensor("wv", [128, KC * WVN], f32r, kind="ExternalInput")
    wo = nc.dram_tensor("wo", [128, HPC * WON], f32r,
                        kind="ExternalInput")
    o_dram = nc.dram_tensor("o", [128, nit * C], f32, kind="ExternalOutput")

    with tile.TileContext(nc) as tc:
        with (
            tc.tile_pool(name="persist", bufs=1) as pp,
            tc.tile_pool(name="ppt", bufs=5) as ppt,
            tc.tile_pool(name="pbcs", bufs=2) as pbcs,
            tc.tile_pool(name="psc_ps", bufs=3, space="PSUM") as psc,
            tc.tile_pool(name="pav_ps", bufs=1, space="PSUM") as pav,
            tc.tile_pool(name="pbc_ps", bufs=1, space="PSUM") as pbc,
        ):
            # ---- persistent tiles ----
            # qT/kT in bf16: halves SBUF so the hsT staging pool can stay
            # open through the whole head-0 attention phase (head-1
            # projections are interleaved into it as PE filler work)
            qT = [pp.tile([128, s], bf16, name=f"qT{h}") for h in range(HPC)]
            kT = [pp.tile([128, s], bf16, name=f"kT{h}") for h in range(HPC)]
            # v tiles per key-tile, per-head stride 97:
            # [head data (80) | zero pad (16) | one] -- ones at 96 so the AV
            # row sum lands on a 32-aligned PSUM partition
            VS = 97
            v_sb = pp.tile([128, njt, 2 * VS], bf16, name="v_sb")
            wo_sb = pp.tile([128, HPC, WON], f32r, name="wo_sb")
            avn = [pp.tile([128, s], f32r, name=f"avn{h}")
                   for h in range(HPC)]
            recip_sb = pp.tile([128, 512], f32r, name="recip_sb")
            ones_sb = pp.tile([128, D], f32r, name="ones_sb")

            # f32r tiles can't be Memset; fill via DVE copy from a broadcast
            # f32 source (the engine cast satisfies the fp32r rounding rule).
            # All on DVE so matmul waits merge on one semaphore.
            zsrc = pp.tile([128, 8], f32, name="zsrc")
            osrc = pp.tile([128, 8], f32, name="osrc")
            nc.vector.memset(zsrc[:], 0.0)
            nc.vector.memset(osrc[:], 1.0)

            def zfill(dst2d):
                nc.vector.tensor_copy(
                    dst2d, zsrc[:dst2d.shape[0], 0:1].broadcast_to(
                        dst2d.shape))

            nc.vector.memset(v_sb[:, :, :], 0.0)
            nc.vector.memset(v_sb[:, :, VS - 1], 1.0)
            nc.vector.memset(v_sb[:, :, 2 * VS - 1], 1.0)
            for h in range(HPC):
                nc.vector.memset(kT[h][:, :], 0.0)
                nc.vector.memset(qT[h][:, :], 0.0)
                zfill(avn[h][:, :])
            zfill(recip_sb[:, :])
            zfill(ones_sb[:, :])
            nc.vector.tensor_copy(
                ones_sb[0:1, :], osrc[0:1, 0:1].broadcast_to([1, D]))

            mul_prev = [None]

            def chunk_body(h, i8, filler_hook=None):
                """Attention for one (head, 512-query chunk)."""
                i0 = i8 * 512
                if mul_prev[0] is not None:
                    # absorb the av-slot WAR (DVE) on a dummy so the first
                    # AV matmul below carries only the ACT wait
                    dum2 = pbc.tile([8, 8], f32, name="dum2", tag="bcslot")
                    dmm = nc.tensor.matmul(
                        dum2[:], ones_sb[0:1, 0:8], ones_sb[0:1, 0:8],
                        start=True, stop=True, skip_group_check=True)
                    add_dep_helper(dmm.ins, mul_prev[0].ins,
                                   reason="absorb av WAR on PE")
                av = pav.tile([VS, 512], f32, name="av_ps")
                for jg in range(njt // 2):
                    if filler_hook is not None and (
                            filler_hook.__name__ == "urgent_hook"
                            or jg % 4 == 3):
                        filler_hook()
                    sc = psc.tile([128, 1024], f32, name="sc_ps", tag="scslot")
                    for jj in range(2):
                        j = 2 * jg + jj
                        nc.tensor.matmul(
                            sc[:, jj * 512:(jj + 1) * 512],
                            kT[h][:, j * 128:(j + 1) * 128],
                            qT[h][:, i0:i0 + 512],
                            start=True, stop=True,
                        )
                    pt = ppt.tile([128, 1024], bf16, name="pt")
                    nc.scalar.activation(
                        out=pt[:], in_=sc[:],
                        func=mybir.ActivationFunctionType.Exp,
                        scale=SCALE,
                    )
                    for jj in range(2):
                        j = 2 * jg + jj
                        nc.tensor.matmul(
                            av[:],
                            v_sb[:, j, h * VS:(h + 1) * VS],
                            pt[:, jj * 512:(jj + 1) * 512],
                            start=(j == 0), stop=(j == njt - 1),
                        )
                # normalize: avn = av[0:D] * (1 / rowsum) broadcast
                with nc.allow_low_precision(
                        reason="fp32r recip feeds broadcast matmul"):
                    nc.vector.reciprocal(recip_sb[0:1, :],
                                         av[VS - 1:VS, :])
                bc = pbc.tile([D, 512], f32, name="bc_ps", tag="bcslot")
                nc.tensor.matmul(bc[:], ones_sb[:], recip_sb[:],
                                 start=True, stop=True)
                av_sb2 = pbcs.tile([D, 512], f32, name="av_sb2")
                nc.vector.tensor_copy(av_sb2[:], av[0:D, :])
                mul_prev[0] = nc.vector.tensor_mul(
                    avn[h][0:D, i0:i0 + 512], av_sb2[:], bc[:])

            # ============ Phase A + head-0 attention (hsT resident) =======
            with (
                tc.tile_pool(name="pA", bufs=1) as pA,
            ):
                hsT_sb = pA.tile([128, KC, s], f32r, name="hsT_sb")
                wq_sb = pA.tile([128, KC, HPC * D], f32r, name="wq_sb")
                wk_sb = pA.tile([128, KC, HPC * D], f32r, name="wk_sb")
                wv_sb = pA.tile([128, KC, WVN], f32r, name="wv_sb")

                # weights first (small, needed by the first matmuls),
                # then hsT in two column-half batches per kc chunk so the
                # first half of the projections can start at ~half DMA time
                nc.sync.dma_start(
                    wk_sb.rearrange("p a b -> p (a b)"), wk[:, :])
                nc.sync.dma_start(
                    wv_sb.rearrange("p a b -> p (a b)"), wv[:, :])
                nc.sync.dma_start(
                    wq_sb.rearrange("p a b -> p (a b)"), wq[:, :])
                nc.sync.dma_start(
                    wo_sb.rearrange("p a b -> p (a b)"), wo[:, :])
                hh = s // 2
                qq = s // 4
                for cb in range(4):
                    for kc in range(KC):
                        nc.sync.dma_start(
                            hsT_sb[:, kc, cb * qq:(cb + 1) * qq],
                            hsT[:, kc * s + cb * qq:kc * s + cb * qq + qq])

                # dummy matmuls: make PE observe every DMA-queue semaphore
                # (fp32r matmuls can carry only one sync wait each);
                # batch-2 dummies are emitted after the first-half work below
                dum = pbc.tile([8, 8], f32, name="dum", tag="bcslot")
                for src in ([wq_sb[0:1, 0, 0:8], wk_sb[0:1, 0, 0:8],
                             wv_sb[0:1, 0, 0:8], wo_sb[0:1, 0, 0:8]] +
                            [hsT_sb[0:1, kc, 0:8] for kc in range(KC)] +
                            [hsT_sb[0:1, kc, qq:qq + 8]
                             for kc in range(KC)]):
                    nc.tensor.matmul(dum[:], src, src, start=True, stop=True,
                                     skip_group_check=True)

                def emit_qk_chunk(h, w_sb, dst, iq):
                    ps = psc.tile([D, 512], f32, name="qk_ps", tag="scslot")
                    for kc in range(KC):
                        nc.tensor.matmul(
                            ps[:],
                            w_sb[:, kc, h * D:(h + 1) * D],
                            hsT_sb[:, kc, iq * 512:(iq + 1) * 512],
                            start=(kc == 0), stop=(kc == KC - 1),
                        )
                    cp = nc.vector.tensor_copy(
                        dst[0:D, iq * 512:(iq + 1) * 512], ps[:])
                    del cp

                def emit_v_tile(jt):
                    ps = psc.tile([128, WVN], f32, name="v_ps", tag="scslot")
                    for kc in range(KC):
                        nc.tensor.matmul(
                            ps[:],
                            hsT_sb[:, kc, jt * 128:(jt + 1) * 128],
                            wv_sb[:, kc, :],
                            start=(kc == 0), stop=(kc == KC - 1),
                        )
                    c0 = nc.vector.tensor_copy(v_sb[:, jt, 0:D], ps[:, 0:D])
                    c1 = nc.vector.tensor_copy(v_sb[:, jt, VS:VS + D],
                                               ps[:, D:2 * D])
                    del c0, c1

                # minimal prefix for head-0 attention: first halves of
                # kT[0] and v (covering key tiles 0..njt/2-1) + qT[0] i0;
                # the second halves are emitted as high-rate fillers inside
                # chunk 0's attention loop (they stay ahead of consumption)
                for iq in range(nch // 2):
                    emit_qk_chunk(0, wk_sb, kT[0], iq)
                for jt in range(njt // 2):
                    emit_v_tile(jt)
                for kc in range(KC):
                    for off in (hh, hh + qq):
                        src2 = hsT_sb[0:1, kc, off:off + 8]
                        nc.tensor.matmul(dum[:], src2, src2, start=True,
                                         stop=True, skip_group_check=True)
                emit_qk_chunk(0, wq_sb, qT[0], 0)

                # remaining projections become PE filler work inside the
                # head-0 attention loop (ACT-paced -> PE has slack there)
                # urgent fillers: second halves of kT[0] / v, interleaved
                # so supply stays ahead of the chunk-0 attention loop's
                # consumption (kT iq covers 4 key tiles, each jg eats 2)
                urgent = []
                vj = njt // 2
                for iq in range(nch // 2, nch):
                    urgent.append(("k0", iq))
                    for _ in range(4):
                        if vj < njt:
                            urgent.append(("v", vj))
                            vj += 1
                while vj < njt:
                    urgent.append(("v", vj))
                    vj += 1
                fillers = (
                    [(0, "q", iq) for iq in range(1, nch)] +
                    [(1, "k", iq) for iq in range(nch)] +
                    [(1, "q", iq) for iq in range(nch)]
                )
                fq = list(fillers)
                emitted = {(0, "q", 0)}

                def filler_hook(n=1):
                    for _ in range(n):
                        if urgent:
                            kind, idx = urgent.pop(0)
                            if kind == "k0":
                                emit_qk_chunk(0, wk_sb, kT[0], idx)
                            else:
                                emit_v_tile(idx)
                        elif fq:
                            h2, t2, iq2 = fq.pop(0)
                            w2 = wq_sb if t2 == "q" else wk_sb
                            d2 = qT[h2] if t2 == "q" else kT[h2]
                            emit_qk_chunk(h2, w2, d2, iq2)
                            emitted.add((h2, t2, iq2))

                def urgent_hook():
                    # 2 per jg: outpaces consumption (2 v-tiles + 0.5 kT
                    # groups per jg, starting 8 jg in) without starving
                    # ScalarE behind a PE filler burst
                    filler_hook(2)

                for i8 in range(nch):
                    # dependency order is EMISSION order: this chunk's qT
                    # slice and all urgent work must precede its consumers
                    if i8 > 0:
                        while urgent:
                            filler_hook()
                    while (0, "q", i8) not in emitted:
                        filler_hook()
                    chunk_body(0, i8,
                               urgent_hook if i8 == 0 else filler_hook)
                while fq or urgent:
                    filler_hook()

            # ============ head-1 attention + output projection ============
            with (
                tc.tile_pool(name="pobuf", bufs=2) as pobuf,
            ):
                o_state = {"buf": None}
                cq = []   # deferred Phase-C i-tile indices

                def emit_c_tile(g):
                    if g % qsize == 0:
                        o_state["buf"] = pobuf.tile([128, qsize, C], f32,
                                                    name="o_buf")
                    o_buf = o_state["buf"]
                    t0 = g * 128
                    o_ps = psc.tile([128, WON], f32, name="o_ps", tag="scslot")
                    for n0, n1 in ((0, 512), (512, WON)):
                        for h in range(HPC):
                            nc.tensor.matmul(
                                o_ps[:, n0:n1],
                                avn[h][:, t0:t0 + 128],
                                wo_sb[:, h, n0:n1],
                                start=(h == 0), stop=(h == HPC - 1),
                            )
                    nc.vector.tensor_copy(o_buf[:, g % qsize, :], o_ps[:, 0:C])
                    if g % qsize == qsize - 1:
                        q = g // qsize
                        nc.sync.dma_start(
                            o_dram[:, q * qsize * C:(q + 1) * qsize * C],
                            o_buf.rearrange("p a b -> p (a b)"),
                        )

                def c_hook():
                    if cq:
                        emit_c_tile(cq.pop(0))

                for i8 in range(nch):
                    # Phase C of the previous chunk interleaves into this
                    # chunk's attention loop (keeps ACT fed at boundaries)
                    chunk_body(1, i8, c_hook if cq else None)
                    while cq:
                        c_hook()
                    cq.extend(range(i8 * 4, i8 * 4 + 4))
                while cq:
                    c_hook()

    nc.compile()
    return nc


def _get_nc(s=S):
    if s not in _NC_CACHE:
        _NC_CACHE[s] = build_nc(s)
    return _NC_CACHE[s]


def _pmajor(a, width):
    """[KC*128, width] -> partition-major [128, KC*width]."""
    kc = a.shape[0] // 128
    return np.ascontiguousarray(
        a.reshape(kc, 128, width).transpose(1, 0, 2).reshape(128, kc * width))


def make_in_maps(hidden_states, Wq, Wk, Wv, Wo, s=S):
    """Shard full inputs into 8 per-core input dicts (partition-major)."""
    hs = np.asarray(hidden_states, dtype=np.float32)
    Wq = np.asarray(Wq, dtype=np.float32)
    Wk = np.asarray(Wk, dtype=np.float32)
    Wv = np.asarray(Wv, dtype=np.float32)
    Wo = np.asarray(Wo, dtype=np.float32)
    hsT = [_pmajor(np.ascontiguousarray(hs[b].T), s) for b in range(B)]
    in_maps = []
    for c in range(NCORES):
        b, hp = divmod(c, NCORES // B)
        rows = slice(HPC * D * hp, HPC * D * (hp + 1))
        wv_t = np.ascontiguousarray(Wv[rows, :].T)          # [C, 160]
        wv_pad = np.zeros((C, WVN), np.float32)
        wv_pad[:, :HPC * D] = wv_t
        wo_t = np.ascontiguousarray(Wo[:, rows].T)          # [160, C]
        wo_pad = np.zeros((HPC, 128, WON), np.float32)
        wo_pad[:, :D, :C] = wo_t.reshape(HPC, D, C)
        wo_pm = np.ascontiguousarray(
            wo_pad.transpose(1, 0, 2).reshape(128, HPC * WON))
        in_maps.append({
            "hsT": hsT[b],
            "wq": _pmajor(np.ascontiguousarray(Wq[rows, :].T), HPC * D),
            "wk": _pmajor(np.ascontiguousarray(Wk[rows, :].T), HPC * D),
            "wv": _pmajor(wv_pad, WVN),
            "wo": wo_pm,
        })
    return in_maps


def unpermute_o(o_core, s=S):
    """[128, (s/128)*C] partition-major -> [s, C]."""
    nit = s // 128
    return o_core.reshape(128, nit, C).transpose(1, 0, 2).reshape(s, C)


def assemble(results, hidden_states, bo):
    hs = np.asarray(hidden_states, dtype=np.float32)
    bo = np.asarray(bo, dtype=np.float32)
    out = np.empty((B, S, C), dtype=np.float32)
    ncb = NCORES // B
    for b in range(B):
        acc = unpermute_o(results[b * ncb]["o"]).astype(np.float64)
        for k in range(1, ncb):
            acc = acc + unpermute_o(results[b * ncb + k]["o"])
        out[b] = (acc + bo[None, :]).astype(np.float32) + hs[b]
    return out


def kernel(hidden_states, Wq, Wk, Wv, Wo, bo):
    from concourse.bass_utils import run_bass_kernel_spmd

    nc = _get_nc(S)
    in_maps = make_in_maps(hidden_states, Wq, Wk, Wv, Wo)
    res = run_bass_kernel_spmd(nc, in_maps, core_ids=list(range(NCORES)))
    return assemble(res.results, hidden_states, bo)



# revision 5
# speedup vs baseline: 1.5683x; 1.5683x over previous
"""CombinedAttentionProcessor kernel for 8 Trainium2 NeuronCores (fp8 version).

Problem: B=2, S=4096, C=640, H=8 heads, D=80 head_dim.
    q/k/v = hs @ W{q,k,v}.T ; per-(b,h): softmax(q k^T / sqrt(D)) v ;
    out = attn @ Wo.T + bo + residual.

Sharding: 16 (batch, head) groups -> 2 per core. Each core computes its 2
heads' attention and a partial output projection [S, C]; the host sums the 4
partials per batch, rescales, and adds bias + residual.

All matmuls run in fp8e4m3 with DoubleRow perf mode (2 contraction subtiles
per instruction, 0.5 cycles/row): weights are host-scaled by 16 so fp8
quantization error stays ~4% relative; the final output is descaled by 1/256
on the host. The softmax exp is split across three engines per key-tile-pair:
ACT computes exp(x)/2 natively; DVE and GPSIMD compute a Schraudolph-style
exp via uint8 = round(score * 8*log2(e)*scale + 48) bitcast to fp8e4m3
(the float->uint8 cast saturates negatives to zero on HW, handling the low
tail). The 1/2 scale bias cancels in the softmax normalization.

Layouts (c = kc*128+p, contraction always on partitions):
  hsT  [128, 8cb, 6kc, 512]   fp8   hs^T column-chunked, kc-tile 5 zero-pad
  wq/wk [128, 6kc, 2h, 2half, 40]  fp8*16
  wv   [128, 6kc, 2h, 80]     fp8*16
  wo   [128(d), 2h, 640]      fp8*16
  o    [128, 32, 640]         bf16  partition-major i-tiles (host descales)
On-chip: qT/kT per head [40p, 2half, S] fp8 (DoubleRow splits D=80 into
2x40); v [128, 32j, 2h, 97] fp8 with ones at col 96 (AV row 96 = softmax
denominator, 32-aligned partition); avn [80, 2h, S] fp8 normalized.
"""
import sys

if "/opt/trn_rl_repo" not in sys.path:
    sys.path.insert(0, "/opt/trn_rl_repo")

import numpy as np

B, S, C = 2, 4096, 640
H, D = 8, 80
HPC = 2          # heads per core
NCORES = 8
KC = 5           # real contraction tiles over C
KC6 = 6          # padded to even for DoubleRow pairs
NCB = 8          # hsT DMA column batches
WSCALE = 16.0    # host weight scale (fp8 dynamic range)
SCALE = 1.0 / float(np.sqrt(D))
SEFF = SCALE / (WSCALE * WSCALE)        # psum score -> true scaled score
SCHA = SEFF * 8.0 / float(np.log(2.0))  # Schraudolph slope (fp8e4m3, m=3)
# Schraudolph bias is embedded in the scores via a constant contraction row
# (qT/kT row 40, half 0, value 48.0 each -> +2304 in every psum score), so
# the uint8 cast input is >= 0 (no negative wrap; low tail clamps via max).
BROW = 48.0
BPSUM = BROW * BROW                     # 9216
# ACT path must encode the same value: exp(seff*psum + EBIAS) == 2^((i-56)/8)
EBIAS = float(-BPSUM * SEFF - (56.0 - BPSUM * SCHA) * np.log(2.0) / 8.0)
VS = 104  # dual-fp8 ldweights: cols % 4 == 0, subtile stride % 16 == 0
ONESCOL = 96

_NC_CACHE = {}


def build_nc(s=S):
    import concourse.bacc as bacc
    import concourse.mybir as mybir
    import concourse.tile as tile
    from concourse.tile import add_dep_helper

    f32 = mybir.dt.float32
    bf16 = mybir.dt.bfloat16
    fp8 = mybir.dt.float8e4
    u8 = mybir.dt.uint8
    DR = mybir.MatmulPerfMode.DoubleRow
    Exp = mybir.ActivationFunctionType.Exp

    njt = s // 128    # key tiles
    nit = s // 128    # output i-tiles
    nch = s // 512    # query chunks
    njg = njt // 2    # key-tile pairs per chunk
    cbw = s // NCB    # hsT column batch width
    assert s % 512 == 0 and njt % 4 == 0

    nc = bacc.Bacc("TRN2", target_bir_lowering=False, debug=False,
                   num_devices=NCORES)

    hsT = nc.dram_tensor("hsT", [128, NCB * KC6 * cbw], fp8,
                         kind="ExternalInput")
    wq = nc.dram_tensor("wq", [128, KC6 * HPC * D], fp8, kind="ExternalInput")
    wk = nc.dram_tensor("wk", [128, KC6 * HPC * D], fp8, kind="ExternalInput")
    wv = nc.dram_tensor("wv", [128, KC6 * HPC * D], fp8, kind="ExternalInput")
    wo = nc.dram_tensor("wo", [128, HPC * C], fp8, kind="ExternalInput")
    qkb = nc.dram_tensor("qkb", [1, 2 * s], fp8, kind="ExternalInput")
    o_dram = nc.dram_tensor("o", [128, nit * C], bf16,
                             kind="ExternalOutput")

    # engine-assignment helpers --------------------------------------------
    # exp halves per chunk: proportional-rate greedy schedule so each
    # engine's exp time per chunk is equal (ACT 612ns, DVE 658, Pool 806)
    # GPSIMD cannot access PSUM -> only ACT and DVE can read scores.
    _counts = {"A": 9, "D": 7}
    _cost = {"A": 1038.0, "D": 1192.0}
    _n16 = sum(_counts.values())
    EXP_PAT16 = []
    _load = {k: 0.0 for k in _counts}
    for _i in range(_n16):
        pick = max(_counts,
                   key=lambda k: (_i + 1) * _counts[k] / _n16
                   - _load[k] / _cost[k])
        _load[pick] += _cost[pick]
        EXP_PAT16.append(pick)
    if EXP_PAT16[0] != "D":
        EXP_PAT16[EXP_PAT16.index("D")] = EXP_PAT16[0]
        EXP_PAT16[0] = "D"
    cp_state = {"i": 0}

    with tile.TileContext(nc) as tc:
        with (
            tc.tile_pool(name="persist", bufs=1) as pp,
            tc.tile_pool(name="ppt", bufs=8) as ppt,
            tc.tile_pool(name="pbcs", bufs=2) as pbcs,
            tc.tile_pool(name="pobuf", bufs=2) as pobuf,
            tc.tile_pool(name="psc_ps", bufs=3, space="PSUM") as psc,
            tc.tile_pool(name="pprj_ps", bufs=1, space="PSUM") as pprj,
            tc.tile_pool(name="pav_ps", bufs=1, space="PSUM") as pav,
        ):
            # ---- persistent tiles ----
            hsT_sb = pp.tile([128, KC6, s], fp8, name="hsT_sb")
            wq_sb = pp.tile([128, KC6, HPC, 2, 40], fp8, name="wq_sb")
            wk_sb = pp.tile([128, KC6, HPC, 2, 40], fp8, name="wk_sb")
            wv_sb = pp.tile([128, KC6, HPC * D], fp8, name="wv_sb")
            wo_sb = pp.tile([128, HPC, C], fp8, name="wo_sb")
            qT = [pp.tile([128, 2, s], fp8, name=f"qT{h}") for h in range(HPC)]
            kT = [pp.tile([128, 2, s], fp8, name=f"kT{h}") for h in range(HPC)]
            v_sb = pp.tile([128, njt, HPC, VS], fp8, name="v_sb")
            avn = pp.tile([128, HPC, s], fp8, name="avn")
            recip_sb = pp.tile([128, 512], bf16, name="recip_sb")
            ones_sb = pp.tile([128, D], bf16, name="ones_sb")
            ebias = pp.tile([128, 1], f32, name="ebias")

            nc.vector.memset(ebias[:], EBIAS)
            nc.gpsimd.memset(recip_sb[:, :], 0.0)
            nc.gpsimd.memset(ones_sb[:, :], 0.0)
            nc.gpsimd.memset(ones_sb[0:1, :], 1.0)
            # v data cols 0:80 come from the projection; only the pad and the
            # denominator ones-column need initialization
            nc.gpsimd.memset(v_sb[:, :, :, D:VS], 0.0)
            nc.gpsimd.memset(v_sb[:, :, :, ONESCOL], 1.0)
            # ---- input DMAs (cb0 first so projections start early) ----
            def dma_cb(cb):
                nc.sync.dma_start(
                    hsT_sb[:, :, cb * cbw:(cb + 1) * cbw],
                    hsT[:, cb * KC6 * cbw:(cb + 1) * KC6 * cbw])

            dma_cb(0)
            nc.sync.dma_start(wk_sb.rearrange("p a b c d -> p (a b c d)"),
                              wk[:, :])
            nc.sync.dma_start(wv_sb.rearrange("p a b -> p (a b)"), wv[:, :])
            nc.sync.dma_start(wq_sb.rearrange("p a b c d -> p (a b c d)"),
                              wq[:, :])
            dma_cb(1)
            dma_cb(2)
            # softmax bias row at partition 40 (via DMA: engines can't
            # start an AP at a non-32-aligned partition)
            qkb_dmas = [nc.sync.dma_start(t[40:41, 0:2, :], qkb[:, :])
                        for t in qT + kT]
            for cb in range(3, NCB):
                dma_cb(cb)
            nc.sync.dma_start(wo_sb.rearrange("p a b -> p (a b)"), wo[:, :])

            def copy_eng(which=None):
                """Rotate copies across engines for balance."""
                if which == "A":
                    return nc.scalar
                if which == "D":
                    return nc.vector
                if which == "P":
                    return nc.gpsimd
                i = cp_state["i"] = cp_state["i"] + 1
                return (nc.scalar, nc.vector)[i % 2]

            def ecopy(eng, dst, src):
                if eng is nc.scalar:
                    eng.copy(dst, src)
                else:
                    eng.tensor_copy(dst, src)

            # ---- projection units (all-fp8 DoubleRow) ----
            prj_state = {"i": 0}

            def pe_observe(src_ap, extra_dep=None):
                """Tiny matmul so PE observes the semaphore guarding
                src_ap (PE matmuls only honor a single sync wait)."""
                dum = pprj.tile([8, 8], f32, name="dum", tag="prj")
                mm = nc.tensor.matmul(dum[:], src_ap, src_ap, start=True,
                                      stop=True, skip_group_check=True)
                if extra_dep is not None:
                    add_dep_helper(mm.ins, extra_dep.ins,
                                   reason="observe DMA sem on PE")

            def prj_tile(shape):
                i = prj_state["i"] = prj_state["i"] + 1
                if i % 4 == 3:
                    return pprj.tile(shape, f32, name="prj_ps", tag="prj")
                return psc.tile(shape, f32, name="prj_ps", tag="scslot")

            def emit_qk_chunk(h, w_sb, dst, iq, ceng=None):
                """dst[0:40, 0:2, iq*512:(iq+1)*512] = head-h projection."""
                i0 = iq * 512
                for half in range(2):
                    ps = prj_tile([40, 512])
                    for p in range(KC6 // 2):
                        nc.tensor.matmul(
                            ps[:],
                            w_sb[:, 2 * p:2 * p + 2, h, half, :],
                            hsT_sb[:, 2 * p:2 * p + 2, i0:i0 + 512],
                            start=(p == 0), stop=(p == KC6 // 2 - 1),
                            perf_mode=DR,
                        )
                    ecopy(copy_eng(ceng),
                          dst[0:40, half, i0:i0 + 512], ps[:])

            def emit_v_tile(jt, ceng=None):
                ps = prj_tile([128, HPC, D])
                for p in range(KC6 // 2):
                    nc.tensor.matmul(
                        ps.rearrange("p a b -> p (a b)"),
                        hsT_sb[:, 2 * p:2 * p + 2, jt * 128:(jt + 1) * 128],
                        wv_sb[:, 2 * p:2 * p + 2, :],
                        start=(p == 0), stop=(p == KC6 // 2 - 1),
                        perf_mode=DR,
                    )
                ecopy(copy_eng(ceng), v_sb[:, jt, 0:2, 0:D], ps[:, :, :])

            # ---- attention: flat jg stream across all chunks ----
            # (no per-chunk pipeline drain: exp engines stay fed across
            # chunk boundaries; AV matmuls trail by AVLAG positions)
            AVLAG = 4
            av_state = {}

            def emit_norm(h, i8):
                """Normalize chunk (h, i8): avn = av[0:D] / av[96]."""
                i0 = i8 * 512
                av = av_state.pop((h, i8))
                with nc.allow_low_precision(
                        reason="bf16 recip feeds broadcast matmul"):
                    nc.vector.reciprocal(recip_sb[0:1, 0:512],
                                         av[ONESCOL:ONESCOL + 1, :])
                av2 = pbcs.tile([D, 512], f32, name="av2")
                nc.vector.tensor_copy(av2[:], av[0:D, :])
                bc = pprj.tile([D, 512], f32, name="bc_ps", tag="prj")
                nc.tensor.matmul(bc[:], ones_sb[:], recip_sb[:, 0:512],
                                 start=True, stop=True)
                nc.vector.tensor_mul(avn[0:D, h, i0:i0 + 512], av2[:],
                                      bc[:])

            def attention_stream(chunks, filler_hook=None):
                pend = []
                n = len(chunks)
                for g in range(n * njg + AVLAG):
                    if g < n * njg:
                        h, i8 = chunks[g // njg]
                        jg = g % njg
                        i0 = i8 * 512
                        if filler_hook is not None and jg % 4 == 3:
                            filler_hook()
                        pt = ppt.tile([128, 2, 512], fp8, name="pt")
                        sc = psc.tile([128, 2, 512], f32, name="sc_ps",
                                      tag="scslot")
                        for jj in range(2):
                            j = 2 * jg + jj
                            nc.tensor.matmul(
                                sc[:, jj, :],
                                kT[h][0:41, 0:2, j * 128:(j + 1) * 128],
                                qT[h][0:41, 0:2, i0:i0 + 512],
                                start=True, stop=True,
                                perf_mode=DR,
                            )
                        if EXP_PAT16[g % 16] == "A":
                            nc.scalar.activation(
                                out=pt.rearrange("p a b -> p (a b)"),
                                in_=sc.rearrange("p a b -> p (a b)"),
                                func=Exp, scale=SEFF, bias=ebias[:],
                            )
                        else:
                            nc.vector.tensor_scalar(
                                pt.rearrange("p a b -> p (a b)").bitcast(u8),
                                sc.rearrange("p a b -> p (a b)"),
                                SCHA, 0.0,
                                op0=mybir.AluOpType.mult,
                                op1=mybir.AluOpType.max,
                            )
                        pend.append((h, i8, jg, pt))
                    if g >= AVLAG:
                        h2, i82, jg2, pt2 = pend.pop(0)
                        if jg2 == 0:
                            av_state[(h2, i82)] = pav.tile(
                                [VS, 512], f32, name="av_ps")
                        nc.tensor.matmul(
                            av_state[(h2, i82)][:],
                            v_sb[:, 2 * jg2:2 * jg2 + 2, h2, 0:VS],
                            pt2[:, :, :],
                            start=(jg2 == 0), stop=(jg2 == njg - 1),
                            perf_mode=DR,
                        )
                        if jg2 == njg - 1:
                            emit_norm(h2, i82)

            # ---- output projection ----
            o_state = {"buf": None}

            def emit_c_tile(g, ceng=None):
                if g % 4 == 0:
                    o_state["buf"] = pobuf.tile([128, 4, C], bf16,
                                                name="o_buf")
                o_buf = o_state["buf"]
                t0 = g * 128
                o_ps = psc.tile([128, C], f32, name="o_ps", tag="scslot")
                for n0, n1 in ((0, 512), (512, C)):
                    for hh in range(HPC):
                        nc.tensor.matmul(
                            o_ps[:, n0:n1], avn[0:D, hh, t0:t0 + 128],
                            wo_sb[0:D, hh, n0:n1],
                            start=(hh == 0), stop=(hh == HPC - 1),
                        )
                ecopy(copy_eng(ceng), o_buf[:, g % 4, :], o_ps[:])
                if g % 4 == 3:
                    nc.sync.dma_start(
                        o_dram[:, (g - 3) * C:(g + 1) * C],
                        o_buf.rearrange("p a b -> p (a b)"))

            # ============ Phase A: all projections ========================
            # PE must observe every input-DMA semaphore once (single-wait
            # rule) before real matmuls depend on them
            seen_cb = set()

            def observe_cb(cb):
                if cb not in seen_cb:
                    seen_cb.add(cb)
                    pe_observe(hsT_sb[0:8, 0, cb * cbw:cb * cbw + 8])

            pe_observe(wk_sb[0:8, 0, 0, 0, 0:8])
            pe_observe(wv_sb[0:8, 0, 0:8])
            pe_observe(wq_sb[0:8, 0, 0, 0, 0:8])
            observe_cb(0)

            def emit_qk_all(h, w_sb, dst, iq):
                for cb in range((iq * 512) // cbw,
                                ((iq + 1) * 512 - 1) // cbw + 1):
                    observe_cb(cb)
                emit_qk_chunk(h, w_sb, dst, iq)

            for iq in range(nch):
                emit_qk_all(0, wk_sb, kT[0], iq)
            for jt in range(njt):
                observe_cb((jt * 128) // cbw)
                emit_v_tile(jt)
            for iq in range(nch):
                emit_qk_all(0, wq_sb, qT[0], iq)
            for iq in range(nch):
                emit_qk_all(1, wk_sb, kT[1], iq)
            for iq in range(nch):
                emit_qk_all(1, wq_sb, qT[1], iq)
            # observe qkb bias rows and the tail projection copies on both
            # engines before attention consumes them
            for dma in qkb_dmas:
                pe_observe(ones_sb[0:8, 0:8], extra_dep=dma)
            pe_observe(qT[1][0:8, 1, s - 8:s])
            pe_observe(qT[1][0:8, 0, s - 8:s])
            pe_observe(kT[1][0:8, 1, s - 8:s])
            pe_observe(kT[1][0:8, 0, s - 8:s])
            pe_observe(qT[0][0:8, 1, s - 8:s])
            pe_observe(kT[0][0:8, 1, s - 8:s])
            pe_observe(v_sb[0:8, njt - 1, 1, 0:8])

            # ============ Phase B: attention (both heads) ==================
            attention_stream([(h, i8) for h in range(HPC)
                              for i8 in range(nch)])

            # ============ Phase C: out-projection ==========================
            pe_observe(wo_sb[0:8, 0, 0:8])
            pe_observe(avn[0:8, 1, s - 8:s])
            for g in range(nit):
                emit_c_tile(g, "D")

    nc.compile()
    return nc


def _get_nc(s=S):
    if s not in _NC_CACHE:
        _NC_CACHE[s] = build_nc(s)
    return _NC_CACHE[s]


def make_in_maps(hidden_states, Wq, Wk, Wv, Wo, s=S):
    """Shard full inputs into 8 per-core fp8 input dicts."""
    import ml_dtypes
    fp8 = ml_dtypes.float8_e4m3

    cbw = s // NCB
    hs = np.asarray(hidden_states, dtype=np.float32)
    Wq = np.asarray(Wq, dtype=np.float32)
    Wk = np.asarray(Wk, dtype=np.float32)
    Wv = np.asarray(Wv, dtype=np.float32)
    Wo = np.asarray(Wo, dtype=np.float32)

    # hsT[p, cb, kc, u] = hs[b][cb*cbw+u, kc*128+p]; kc=5 zero
    hsTs = []
    for b in range(B):
        t = hs[b].T.reshape(KC, 128, NCB, cbw)  # [kc, p, cb, u]
        hp8 = np.zeros((128, NCB, KC6, cbw), np.float32)
        hp8[:, :, :KC, :] = t.transpose(1, 2, 0, 3)
        hsTs.append(hp8.reshape(128, NCB * KC6 * cbw).astype(fp8))

    def pack_qk(W, hp):
        # -> [128, KC6, HPC, 2, 40]
        out = np.zeros((128, KC6, HPC, 2, 40), np.float32)
        rows = W[HPC * D * hp:HPC * D * (hp + 1), :] * WSCALE  # [160, C]
        r = rows.reshape(HPC, 2, 40, KC, 128)
        out[:, :KC] = r.transpose(4, 3, 0, 1, 2)
        return np.ascontiguousarray(
            out.reshape(128, KC6 * HPC * D)).astype(fp8)

    def pack_v(W, hp):
        out = np.zeros((128, KC6, HPC * D), np.float32)
        rows = W[HPC * D * hp:HPC * D * (hp + 1), :] * WSCALE  # [160, C]
        r = rows.reshape(HPC * D, KC, 128)
        out[:, :KC] = r.transpose(2, 1, 0)
        return np.ascontiguousarray(out.reshape(128, KC6 * HPC * D)).astype(fp8)

    def pack_wo(W, hp):
        # wo[p(d), h, c] = 16*Wo[c, hp*160 + h*80 + p]
        out = np.zeros((128, HPC, C), np.float32)
        cols = W[:, HPC * D * hp:HPC * D * (hp + 1)] * WSCALE  # [C, 160]
        out[0:D] = cols.T.reshape(HPC, D, C).transpose(1, 0, 2)
        return np.ascontiguousarray(out.reshape(128, HPC * C)).astype(fp8)

    in_maps = []
    for c in range(NCORES):
        b, hp = divmod(c, NCORES // B)
        qkb = np.zeros((1, 2 * s), np.float32)
        qkb[0, :s] = BROW
        in_maps.append({
            "hsT": hsTs[b],
            "qkb": qkb.astype(fp8),
            "wq": pack_qk(Wq, hp),
            "wk": pack_qk(Wk, hp),
            "wv": pack_v(Wv, hp),
            "wo": pack_wo(Wo, hp),
        })
    return in_maps


def unpermute_o(o_core, s=S):
    """[128, (s/128)*C] partition-major bf16 -> [s, C] f32."""
    nit = s // 128
    return np.asarray(o_core, dtype=np.float32).reshape(
        128, nit, C).transpose(1, 0, 2).reshape(s, C)


def assemble(results, hidden_states, bo, s=S):
    hs = np.asarray(hidden_states, dtype=np.float32)
    bo = np.asarray(bo, dtype=np.float32)
    out = np.empty((B, s, C), dtype=np.float32)
    ncb = NCORES // B
    descale = 1.0 / (WSCALE * WSCALE)
    for b in range(B):
        acc = unpermute_o(results[b * ncb]["o"], s).astype(np.float64)
        for k in range(1, ncb):
            acc = acc + unpermute_o(results[b * ncb + k]["o"], s)
        out[b] = (acc * descale + bo[None, :]).astype(np.float32) + hs[b]
    return out


def kernel(hidden_states, Wq, Wk, Wv, Wo, bo):
    from concourse.bass_utils import run_bass_kernel_spmd

    nc = _get_nc(S)
    in_maps = make_in_maps(hidden_states, Wq, Wk, Wv, Wo)
    res = run_bass_kernel_spmd(nc, in_maps, core_ids=list(range(NCORES)))
    return assemble(res.results, hidden_states, bo)
